# revision 45
# baseline (speedup 1.0000x reference)
"""Trainium2 Bass kernel for a 2-layer GCN over 2048 independent 25-node
KNN subgraphs (gnn_message_passing).

Strategy (v3, aggregate-first + software-pipelined emission):
  - Each 25-node subgraph is independent -> the sparse aggregation is a
    dense per-graph 25x25 matmul. Host packs the normalized adjacency
    into block-diagonal 128x128 tiles (5 graphs per tile, rows/cols
    125..127 zero), bf16 everywhere (rel err ~8e-3 << 2e-2 budget).
  - Layer 1 is computed aggregation-first with a feature-major
    intermediate so every matmul keeps a 128x128 bf16 stationary (FWL):
        zT  = x_tile.T @ at_tile     (stationary = x, moving = at)
        h1  = relu(zT.T @ W0)        (stationary = zT, moving = W0)
    zT's PSUM->SBUF copy moves half the bytes a transform-first q would.
  - Layer-2 aggregation needs only the 5 centers/tile: two tiny matmuls
    (stationary = h1 chunks, moving = atc [128,8]) accumulate into a
    PSUM bank that persists for a 13-tile block; one copy per block.
  - W1 + Wlin run per 13-tile block, deferred 1-2 batches so their
    dependency chain never head-of-line-blocks the Tensor stream.
  - Elementwise PSUM->SBUF traffic batches 4 tiles per instruction and
    splits between Vector and Scalar engines.
  - Emission is software-pipelined: mmA(b+1) is emitted before mmB(b)
    so the zT cast latency hides behind independent matmuls.
  - Data parallel over 8 cores: 256 graphs (52 tiles) per core.
"""

import os
import sys

import ml_dtypes
import numpy as np

for _p in ("/opt/trn_rl_repo", "/opt/trn_rl_repo/concourse"):
    if _p not in sys.path:
        sys.path.insert(0, _p)

import concourse.bass as bass
import concourse.tile as tile
from concourse import bacc, mybir
from concourse.bass_utils import run_bass_kernel_spmd

NCORES = 8
B = 2048            # graphs
K = 25              # nodes per graph
N = B * K           # 51200
GPC = B // NCORES   # 256 graphs per core
G = 5               # graphs packed per PE tile
P = G * K           # 125 real partitions per tile
PP = 128            # padded partition count (FWL wants full 128)
NT = (GPC + G - 1) // G   # 52 tiles per core (last tile: 1 real graph)
CP = 8              # padded center count per tile
AW = 128            # adjacency tile width (125 block cols + 3 zero pad)
F0 = 128            # input features
F1 = 256            # hidden features
TB = 4              # tiles per elementwise batch
NB = NT // TB       # 13 batches
BB = [0, 14, 28, 42, 47, 52]   # output block bounds (last blocks short
                               # so the exposed tail epilogue is small)
VSPLIT = 32         # relu cols done on vector engine (rest on scalar)
NWARM = 16          # PE warm-up matmuls (HAM clock gate opens after up
                    # to ~2 free-running 3.4us activity windows; run
                    # dummies during the DMA head so real matmuls start
                    # warm)

_f32 = mybir.dt.float32
_bf16 = mybir.dt.bfloat16

_compiled = {}


def _build_nc():
    nc = bacc.Bacc("TRN2", target_bir_lowering=False, debug=False,
                   num_devices=NCORES)

    x_d = nc.dram_tensor("x", [PP, NT, F0], _bf16, kind="ExternalInput")
    at_d = nc.dram_tensor("at", [PP, NT, AW], _bf16, kind="ExternalInput")
    atc_d = nc.dram_tensor("atc", [PP, NT, CP], _bf16, kind="ExternalInput")
    w0_d = nc.dram_tensor("w0", [F0, F1], _bf16, kind="ExternalInput")
    w1_d = nc.dram_tensor("w1", [128, 2, F1], _bf16, kind="ExternalInput")
    wl_d = nc.dram_tensor("wl", [128, 2], _bf16, kind="ExternalInput")
    out_d = nc.dram_tensor("out", [1, NT * CP], _f32, kind="ExternalOutput")

    relu = mybir.ActivationFunctionType.Relu

    with tile.TileContext(nc) as tc:
        with (
            tc.tile_pool(name="const", bufs=1) as cpool,
            tc.tile_pool(name="ztp", bufs=2) as ztp,
            tc.tile_pool(name="h1p", bufs=3) as h1p,
            tc.tile_pool(name="p2p", bufs=2) as p2p,
            tc.tile_pool(name="h3p", bufs=2) as h3p,
            tc.tile_pool(name="outp", bufs=1) as outp,
            tc.tile_pool(name="psum", bufs=1, space=bass.MemorySpace.PSUM) as psp,
        ):
            # ---- resident inputs; tile 0's deps issue first, on the
            # scalar engine's DMA queue (it initializes earliest and is
            # otherwise idle until the first relu) ----
            w0 = cpool.tile([F0, F1], _bf16)
            x_sb = cpool.tile([PP, NT, F0], _bf16)
            at_sb = cpool.tile([PP, NT, AW], _bf16)
            atc_sb = cpool.tile([PP, NT, CP], _bf16)
            w1 = cpool.tile([128, 2, F1], _bf16)
            wl = cpool.tile([128, 2], _bf16)
            out_sb = outp.tile([1, NT * CP], _f32)

            # scratch for PE warm-up, memset first on the otherwise-idle
            # gpsimd engine so the warm-up matmuls start right after the
            # framework's init barrier
            scratch = cpool.tile([128, 512], _bf16)
            nc.gpsimd.memset(scratch[:], 0.0)

            # Only sync (SP) and scalar (Activation) have hardware DGE
            # rings; gpsimd DMAs take the slow software path. Put all bulk
            # x/at traffic on the two HW rings, balanced so scalar's issue
            # work finishes before its first relu; late-needed weights ride
            # the slow gpsimd path.
            nc.scalar.dma_start(x_sb[:, 0:2, :], x_d[:, 0:2, :])
            nc.sync.dma_start(at_sb[:, 0:2, :], at_d[:, 0:2, :])
            nc.scalar.dma_start(w0[:], w0_d[:])
            nc.gpsimd.dma_start(atc_sb[:], atc_d[:])
            bounds = [2, 4, 8, 12, 18, 26, 36, 52]
            for c in range(len(bounds) - 1):
                lo, hi = bounds[c], bounds[c + 1]
                nc.sync.dma_start(x_sb[:, lo:hi, :], x_d[:, lo:hi, :])
                if c < 4:
                    nc.scalar.dma_start(at_sb[:, lo:hi, :],
                                        at_d[:, lo:hi, :])
                else:
                    nc.sync.dma_start(at_sb[:, lo:hi, :], at_d[:, lo:hi, :])
            nc.gpsimd.dma_start(w1[:], w1_d[:])
            nc.gpsimd.dma_start(wl[:], wl_d[:])

            # ---- PE warm-up: dummy matmuls on scratch during the DMA head
            warm_ps = psp.tile([128, 512], _f32, tag="fin", bufs=1)
            for _ in range(NWARM):
                nc.tensor.matmul(warm_ps[:], scratch[:, 0:128], scratch[:],
                                 start=True, stop=True)

            # ---- software-pipelined main loop ----
            state = {"p2_ps": None}
            pending = {}

            def defer(b, fn):
                pending.setdefault(b, []).append(fn)

            def emit_mma(b):
                zt_ps = psp.tile([128, TB, 128], _f32, tag="zt", bufs=2)
                for j in range(TB):
                    i = b * TB + j
                    nc.tensor.matmul(zt_ps[:, j, :], x_sb[:, i, :],
                                     at_sb[:, i, :], start=True, stop=True)
                return zt_ps

            def emit_w1(blk, p2_sb, bsz):
                def fn():
                    h3_ps = psp.tile([128, 2, bsz * CP], _f32, tag="fin",
                                     bufs=1, name="h3_ps")
                    for foc in range(2):
                        for fic in range(2):
                            nc.tensor.matmul(
                                h3_ps[:, foc, :],
                                w1[:, fic, foc * 128:(foc + 1) * 128],
                                p2_sb[:, fic, :, :],
                                start=(fic == 0), stop=(fic == 1))
                    h3_sb = h3p.tile([128, 2, bsz * CP], _bf16)
                    nc.scalar.activation(h3_sb[:], h3_ps[:], relu)
                    return h3_sb
                return fn

            def emit_out(blk, get_h3, bsz):
                def fn():
                    h3_sb = get_h3()
                    o_ps = psp.tile([1, bsz * CP], _f32, tag="fin", bufs=1,
                                    name="o_ps")
                    for foc in range(2):
                        nc.tensor.matmul(o_ps[:], wl[:, foc:foc + 1],
                                         h3_sb[:, foc, :],
                                         start=(foc == 0), stop=(foc == 1))
                    nc.vector.tensor_copy(
                        out_sb[:, BB[blk] * CP:BB[blk + 1] * CP],
                        o_ps[:])
                return fn

            def emit_reluv_p2(pb, h1_ps, h1_sb, at_b):
                # vector half of batch pb's relu (deferred one batch so
                # vector's casts are never stuck behind it)
                nc.vector.tensor_scalar_max(h1_sb[:, :, 0:VSPLIT],
                                            h1_ps[:, :, 0:VSPLIT], 0.0)
                # L2 center aggregation into the block's persistent bank
                for j in range(TB):
                    i = pb * TB + j
                    if i in BB:
                        state["blk"] = BB.index(i)
                        state["p2_ps"] = psp.tile([128, 2, 16, CP], _f32,
                                                  tag="p2", bufs=1,
                                                  name="p2_ps")
                    blk = state["blk"]
                    ib = i - BB[blk]
                    for fic in range(2):
                        nc.tensor.matmul(
                            state["p2_ps"][:, fic, ib, :],
                            h1_sb[:, j, fic * 128:(fic + 1) * 128],
                            atc_sb[:, i, :],
                            start=True, stop=True)

                    if i + 1 == BB[blk + 1]:
                        bsz = BB[blk + 1] - BB[blk]
                        p2_sb = p2p.tile([128, 2, bsz, CP], _bf16)
                        nc.vector.tensor_copy(p2_sb[:],
                                              state["p2_ps"][:, :, 0:bsz, :])
                        w1fn = emit_w1(blk, p2_sb, bsz)
                        cell = {}

                        def run_w1(w1fn=w1fn, cell=cell):
                            cell["h3"] = w1fn()

                        defer(at_b + 1, run_w1)
                        defer(at_b + 2, emit_out(blk, (lambda cell=cell:
                                                       cell["h3"]), bsz))

            zt_ps = emit_mma(0)
            for b in range(NB):
                # PSUM -> SBUF cast of this batch's zT (vector)
                zt_sb = ztp.tile([128, TB, 128], _bf16)
                nc.vector.tensor_copy(zt_sb[:], zt_ps[:])

                # next batch's aggregation fills the cast gap on Tensor
                if b + 1 < NB:
                    zt_ps = emit_mma(b + 1)

                # L1 transform: h1 = relu(zT.T @ W0)  (node-major)
                h1_ps = psp.tile([128, TB, F1], _f32, tag="h1", bufs=2)
                for j in range(TB):
                    nc.tensor.matmul(h1_ps[:, j, :], zt_sb[:, j, :], w0[:],
                                     start=True, stop=True)
                h1_sb = h1p.tile([128, TB, F1], _bf16)
                nc.scalar.activation(h1_sb[:, :, VSPLIT:F1],
                                     h1_ps[:, :, VSPLIT:F1], relu)
                emit_reluv_p2(b, h1_ps, h1_sb, b)

                # mid-kernel HAM guard: tiny filler matmuls into the p2
                # bank's never-read slot 15 keep the PE activity window
                # alive if the supply stream hiccups (~30ns each when
                # not needed; a re-throttle costs microseconds)
                if 3 <= b <= 7:
                    for _ in range(2):
                        nc.tensor.matmul(state["p2_ps"][:, 0, 15, :],
                                         scratch[0:PP, 0:128],
                                         scratch[0:PP, 0:CP],
                                         start=True, stop=True)

                # flush deferred epilogue work assigned to this batch
                for fn in pending.pop(b, []):
                    fn()

            for b in sorted(list(pending)):
                for fn in pending.pop(b):
                    fn()

            nc.sync.dma_start(out_d[:], out_sb[:])

    nc.compile()
    return nc


def _get_nc(mode=None):
    if "v3" not in _compiled:
        _compiled["v3"] = _build_nc()
    return _compiled["v3"]


def _host_prep(x, edge_weight, W0, W1, Wlin, edge_index):
    bf = ml_dtypes.bfloat16
    src = edge_index[0].astype(np.int64)
    tgt = edge_index[1].astype(np.int64)
    b = src // K
    sl = src - b * K
    tl = tgt - (tgt // K) * K

    # dense raw adjacency per graph, indexed [b, t, s]
    idx = (b * K + tl) * K + sl
    Araw = np.bincount(idx, weights=edge_weight.astype(np.float64),
                       minlength=B * K * K).astype(np.float32).reshape(B, K, K)
    deg = Araw.sum(axis=2)                      # weighted in-degree [B, K]
    with np.errstate(divide="ignore"):
        dinv = np.where(deg > 0, 1.0 / np.sqrt(deg), 0.0).astype(np.float32)
    An = Araw * dinv[:, :, None] * dinv[:, None, :]   # [b, t, s]
    ATn = np.ascontiguousarray(An.transpose(0, 2, 1))  # [b, s, t]

    # scatter graphs into per-core padded tile slots
    SLOTS = NT * G
    ATs = np.zeros((NCORES, SLOTS, K, K), np.float32)
    ATs[:, :GPC] = ATn.reshape(NCORES, GPC, K, K)
    ATs = ATs.reshape(NCORES, NT, G, K, K)

    at = np.zeros((NCORES, NT, PP, AW), np.float32)
    bd = at[:, :, :P, :P].reshape(NCORES, NT, G, K, G, K)
    atc = np.zeros((NCORES, NT, PP, CP), np.float32)
    cent = atc[:, :, :P, :G].reshape(NCORES, NT, G, K, G)
    for g in range(G):
        bd[:, :, g, :, g, :] = ATs[:, :, g]          # block-diagonal AT
        cent[:, :, g, :, g] = ATs[:, :, g, :, 0]     # center (t_local=0) col
    # device layout [PP, NT, .]
    at = np.ascontiguousarray(at.transpose(0, 2, 1, 3).astype(bf))
    atc = np.ascontiguousarray(atc.transpose(0, 2, 1, 3).astype(bf))

    # node-major x, tiled and padded: x_nm[p, i, f] = x[i*P + p, f], p < 125
    xp = np.zeros((NCORES, NT, PP, F0), np.float32)
    xtmp = np.zeros((NCORES, NT * P, F0), np.float32)
    xtmp[:, :GPC * K] = x.reshape(NCORES, GPC * K, F0)
    xp[:, :, :P, :] = xtmp.reshape(NCORES, NT, P, F0)
    x_nm = np.ascontiguousarray(xp.transpose(0, 2, 1, 3).astype(bf))

    w1p = np.ascontiguousarray(
        W1.reshape(2, 128, F1).transpose(1, 0, 2).astype(bf))  # [128, fic, fo]
    wl = np.ascontiguousarray(Wlin.reshape(2, 128).T.astype(bf))  # [128, foc]

    in_maps = []
    for c in range(NCORES):
        in_maps.append({
            "x": x_nm[c],
            "at": np.ascontiguousarray(at[c]),
            "atc": np.ascontiguousarray(atc[c]),
            "w0": np.ascontiguousarray(W0.astype(bf)),
            "w1": w1p,
            "wl": wl,
        })
    return in_maps


def _run(inputs, mode=None, trace=False):
    nc = _get_nc()
    in_maps = _host_prep(**inputs)
    res = run_bass_kernel_spmd(nc, in_maps, core_ids=list(range(NCORES)),
                               trace=trace)
    out = np.empty((B, 1), np.float32)
    for c in range(NCORES):
        vals = res.results[c]["out"].reshape(NT, CP)[:, :G].reshape(-1)
        out[c * GPC:(c + 1) * GPC, 0] = vals[:GPC]
    return out, res


def kernel(**inputs):
    out, _ = _run(inputs, trace=False)
    return out


# revision 48
# speedup vs baseline: 1.0073x; 1.0073x over previous
"""Trainium2 Bass kernel for a 2-layer GCN over 2048 independent 25-node
KNN subgraphs (gnn_message_passing).

Strategy (v3, aggregate-first + software-pipelined emission):
  - Each 25-node subgraph is independent -> the sparse aggregation is a
    dense per-graph 25x25 matmul. Host packs the normalized adjacency
    into block-diagonal 128x128 tiles (5 graphs per tile, rows/cols
    125..127 zero), bf16 everywhere (rel err ~8e-3 << 2e-2 budget).
  - Layer 1 is computed aggregation-first with a feature-major
    intermediate so every matmul keeps a 128x128 bf16 stationary (FWL):
        zT  = x_tile.T @ at_tile     (stationary = x, moving = at)
        h1  = relu(zT.T @ W0)        (stationary = zT, moving = W0)
    zT's PSUM->SBUF copy moves half the bytes a transform-first q would.
  - Layer-2 aggregation needs only the 5 centers/tile: two tiny matmuls
    (stationary = h1 chunks, moving = atc [128,8]) accumulate into a
    PSUM bank that persists for a 13-tile block; one copy per block.
  - W1 + Wlin run per 13-tile block, deferred 1-2 batches so their
    dependency chain never head-of-line-blocks the Tensor stream.
  - Elementwise PSUM->SBUF traffic batches 4 tiles per instruction and
    splits between Vector and Scalar engines.
  - Emission is software-pipelined: mmA(b+1) is emitted before mmB(b)
    so the zT cast latency hides behind independent matmuls.
  - Data parallel over 8 cores: 256 graphs (52 tiles) per core.
"""

import os
import sys

import ml_dtypes
import numpy as np

for _p in ("/opt/trn_rl_repo", "/opt/trn_rl_repo/concourse"):
    if _p not in sys.path:
        sys.path.insert(0, _p)

import concourse.bass as bass
import concourse.tile as tile
from concourse import bacc, mybir
from concourse.bass_utils import run_bass_kernel_spmd

NCORES = 8
B = 2048            # graphs
K = 25              # nodes per graph
N = B * K           # 51200
GPC = B // NCORES   # 256 graphs per core
G = 5               # graphs packed per PE tile
P = G * K           # 125 real partitions per tile
PP = 128            # padded partition count (FWL wants full 128)
NT = (GPC + G - 1) // G   # 52 tiles per core (last tile: 1 real graph)
CP = 8              # padded center count per tile
AW = 128            # adjacency tile width (125 block cols + 3 zero pad)
F0 = 128            # input features
F1 = 256            # hidden features
TB = 4              # tiles per elementwise batch
NB = NT // TB       # 13 batches
BB = [0, 14, 28, 42, 49, 52]   # output block bounds (last block tiny
                               # so the exposed tail epilogue is small)
VSPLIT = 32         # relu cols done on vector engine (rest on scalar)
NWARM = 16          # PE warm-up matmuls (HAM clock gate opens after up
                    # to ~2 free-running 3.4us activity windows; run
                    # dummies during the DMA head so real matmuls start
                    # warm)

_f32 = mybir.dt.float32
_bf16 = mybir.dt.bfloat16

_compiled = {}


def _build_nc():
    nc = bacc.Bacc("TRN2", target_bir_lowering=False, debug=False,
                   num_devices=NCORES)

    x_d = nc.dram_tensor("x", [PP, NT, F0], _bf16, kind="ExternalInput")
    at_d = nc.dram_tensor("at", [PP, NT, AW], _bf16, kind="ExternalInput")
    atc_d = nc.dram_tensor("atc", [PP, NT, CP], _bf16, kind="ExternalInput")
    w0_d = nc.dram_tensor("w0", [F0, F1], _bf16, kind="ExternalInput")
    w1_d = nc.dram_tensor("w1", [128, 2, F1], _bf16, kind="ExternalInput")
    wl_d = nc.dram_tensor("wl", [128, 2], _bf16, kind="ExternalInput")
    out_d = nc.dram_tensor("out", [1, NT * CP], _f32, kind="ExternalOutput")

    relu = mybir.ActivationFunctionType.Relu

    with tile.TileContext(nc) as tc:
        with (
            tc.tile_pool(name="const", bufs=1) as cpool,
            tc.tile_pool(name="ztp", bufs=2) as ztp,
            tc.tile_pool(name="h1p", bufs=3) as h1p,
            tc.tile_pool(name="p2p", bufs=2) as p2p,
            tc.tile_pool(name="h3p", bufs=2) as h3p,
            tc.tile_pool(name="outp", bufs=1) as outp,
            tc.tile_pool(name="psum", bufs=1, space=bass.MemorySpace.PSUM) as psp,
        ):
            # ---- resident inputs; tile 0's deps issue first, on the
            # scalar engine's DMA queue (it initializes earliest and is
            # otherwise idle until the first relu) ----
            w0 = cpool.tile([F0, F1], _bf16)
            x_sb = cpool.tile([PP, NT, F0], _bf16)
            at_sb = cpool.tile([PP, NT, AW], _bf16)
            atc_sb = cpool.tile([PP, NT, CP], _bf16)
            w1 = cpool.tile([128, 2, F1], _bf16)
            wl = cpool.tile([128, 2], _bf16)
            out_sb = outp.tile([1, NT * CP], _f32)

            # scratch for PE warm-up, memset first on the otherwise-idle
            # gpsimd engine so the warm-up matmuls start right after the
            # framework's init barrier
            scratch = cpool.tile([128, 512], _bf16)
            nc.gpsimd.memset(scratch[:], 0.0)

            # Only sync (SP) and scalar (Activation) have hardware DGE
            # rings; gpsimd DMAs take the slow software path. Put all bulk
            # x/at traffic on the two HW rings, balanced so scalar's issue
            # work finishes before its first relu; late-needed weights ride
            # the slow gpsimd path.
            nc.scalar.dma_start(x_sb[:, 0:2, :], x_d[:, 0:2, :])
            nc.sync.dma_start(at_sb[:, 0:2, :], at_d[:, 0:2, :])
            nc.scalar.dma_start(w0[:], w0_d[:])
            nc.gpsimd.dma_start(atc_sb[:], atc_d[:])
            bounds = [2, 4, 8, 12, 18, 26, 36, 52]
            for c in range(len(bounds) - 1):
                lo, hi = bounds[c], bounds[c + 1]
                nc.sync.dma_start(x_sb[:, lo:hi, :], x_d[:, lo:hi, :])
                if c < 4:
                    nc.scalar.dma_start(at_sb[:, lo:hi, :],
                                        at_d[:, lo:hi, :])
                else:
                    nc.sync.dma_start(at_sb[:, lo:hi, :], at_d[:, lo:hi, :])
            nc.gpsimd.dma_start(w1[:], w1_d[:])
            nc.gpsimd.dma_start(wl[:], wl_d[:])

            # ---- PE warm-up: dummy matmuls on scratch during the DMA head
            warm_ps = psp.tile([128, 512], _f32, tag="fin", bufs=1)
            for _ in range(NWARM):
                nc.tensor.matmul(warm_ps[:], scratch[:, 0:128], scratch[:],
                                 start=True, stop=True)

            # ---- software-pipelined main loop ----
            state = {"p2_ps": None}
            pending = {}

            def defer(b, fn):
                pending.setdefault(b, []).append(fn)

            def emit_mma(b):
                zt_ps = psp.tile([128, TB, 128], _f32, tag="zt", bufs=2)
                for j in range(TB):
                    i = b * TB + j
                    nc.tensor.matmul(zt_ps[:, j, :], x_sb[:, i, :],
                                     at_sb[:, i, :], start=True, stop=True)
                return zt_ps

            def emit_w1(blk, p2_sb, bsz):
                def fn():
                    h3_ps = psp.tile([128, 2, bsz * CP], _f32, tag="fin",
                                     bufs=1, name="h3_ps")
                    for foc in range(2):
                        for fic in range(2):
                            nc.tensor.matmul(
                                h3_ps[:, foc, :],
                                w1[:, fic, foc * 128:(foc + 1) * 128],
                                p2_sb[:, fic, :, :],
                                start=(fic == 0), stop=(fic == 1))
                    h3_sb = h3p.tile([128, 2, bsz * CP], _bf16)
                    nc.scalar.activation(h3_sb[:], h3_ps[:], relu)
                    return h3_sb
                return fn

            def emit_out(blk, get_h3, bsz):
                def fn():
                    h3_sb = get_h3()
                    o_ps = psp.tile([1, bsz * CP], _f32, tag="fin", bufs=1,
                                    name="o_ps")
                    for foc in range(2):
                        nc.tensor.matmul(o_ps[:], wl[:, foc:foc + 1],
                                         h3_sb[:, foc, :],
                                         start=(foc == 0), stop=(foc == 1))
                    nc.vector.tensor_copy(
                        out_sb[:, BB[blk] * CP:BB[blk + 1] * CP],
                        o_ps[:])
                    # stream each block's output out immediately so only
                    # the tiny final block's DMA sits on the tail
                    nc.sync.dma_start(
                        out_d[:, BB[blk] * CP:BB[blk + 1] * CP],
                        out_sb[:, BB[blk] * CP:BB[blk + 1] * CP])
                return fn

            def emit_reluv_p2(pb, h1_ps, h1_sb, at_b):
                # vector half of batch pb's relu (deferred one batch so
                # vector's casts are never stuck behind it)
                nc.vector.tensor_scalar_max(h1_sb[:, :, 0:VSPLIT],
                                            h1_ps[:, :, 0:VSPLIT], 0.0)
                # L2 center aggregation into the block's persistent bank
                for j in range(TB):
                    i = pb * TB + j
                    if i in BB:
                        state["blk"] = BB.index(i)
                        state["p2_ps"] = psp.tile([128, 2, 16, CP], _f32,
                                                  tag="p2", bufs=1,
                                                  name="p2_ps")
                    blk = state["blk"]
                    ib = i - BB[blk]
                    for fic in range(2):
                        nc.tensor.matmul(
                            state["p2_ps"][:, fic, ib, :],
                            h1_sb[:, j, fic * 128:(fic + 1) * 128],
                            atc_sb[:, i, :],
                            start=True, stop=True)

                    if i + 1 == BB[blk + 1]:
                        bsz = BB[blk + 1] - BB[blk]
                        p2_sb = p2p.tile([128, 2, bsz, CP], _bf16)
                        nc.vector.tensor_copy(p2_sb[:],
                                              state["p2_ps"][:, :, 0:bsz, :])
                        w1fn = emit_w1(blk, p2_sb, bsz)
                        cell = {}

                        def run_w1(w1fn=w1fn, cell=cell):
                            cell["h3"] = w1fn()

                        defer(at_b + 1, run_w1)
                        defer(at_b + 2, emit_out(blk, (lambda cell=cell:
                                                       cell["h3"]), bsz))

            zt_ps = emit_mma(0)
            for b in range(NB):
                # PSUM -> SBUF cast of this batch's zT (vector)
                zt_sb = ztp.tile([128, TB, 128], _bf16)
                nc.vector.tensor_copy(zt_sb[:], zt_ps[:])

                # next batch's aggregation fills the cast gap on Tensor
                if b + 1 < NB:
                    zt_ps = emit_mma(b + 1)

                # L1 transform: h1 = relu(zT.T @ W0)  (node-major)
                h1_ps = psp.tile([128, TB, F1], _f32, tag="h1", bufs=2)
                for j in range(TB):
                    nc.tensor.matmul(h1_ps[:, j, :], zt_sb[:, j, :], w0[:],
                                     start=True, stop=True)
                h1_sb = h1p.tile([128, TB, F1], _bf16)
                nc.scalar.activation(h1_sb[:, :, VSPLIT:F1],
                                     h1_ps[:, :, VSPLIT:F1], relu)
                emit_reluv_p2(b, h1_ps, h1_sb, b)

                # mid-kernel HAM guard: tiny filler matmuls into the p2
                # bank's never-read slot 15 keep the PE activity window
                # alive if the supply stream hiccups (~30ns each when
                # not needed; a re-throttle costs microseconds)
                if 3 <= b <= 7:
                    for _ in range(2):
                        nc.tensor.matmul(state["p2_ps"][:, 0, 15, :],
                                         scratch[0:PP, 0:128],
                                         scratch[0:PP, 0:CP],
                                         start=True, stop=True)

                # flush deferred epilogue work assigned to this batch
                for fn in pending.pop(b, []):
                    fn()

            for b in sorted(list(pending)):
                for fn in pending.pop(b):
                    fn()



    nc.compile()
    return nc


def _get_nc(mode=None):
    if "v3" not in _compiled:
        _compiled["v3"] = _build_nc()
    return _compiled["v3"]


def _host_prep(x, edge_weight, W0, W1, Wlin, edge_index):
    bf = ml_dtypes.bfloat16
    src = edge_index[0].astype(np.int64)
    tgt = edge_index[1].astype(np.int64)
    b = src // K
    sl = src - b * K
    tl = tgt - (tgt // K) * K

    # dense raw adjacency per graph, indexed [b, t, s]
    idx = (b * K + tl) * K + sl
    Araw = np.bincount(idx, weights=edge_weight.astype(np.float64),
                       minlength=B * K * K).astype(np.float32).reshape(B, K, K)
    deg = Araw.sum(axis=2)                      # weighted in-degree [B, K]
    with np.errstate(divide="ignore"):
        dinv = np.where(deg > 0, 1.0 / np.sqrt(deg), 0.0).astype(np.float32)
    An = Araw * dinv[:, :, None] * dinv[:, None, :]   # [b, t, s]
    ATn = np.ascontiguousarray(An.transpose(0, 2, 1))  # [b, s, t]

    # scatter graphs into per-core padded tile slots
    SLOTS = NT * G
    ATs = np.zeros((NCORES, SLOTS, K, K), np.float32)
    ATs[:, :GPC] = ATn.reshape(NCORES, GPC, K, K)
    ATs = ATs.reshape(NCORES, NT, G, K, K)

    at = np.zeros((NCORES, NT, PP, AW), np.float32)
    bd = at[:, :, :P, :P].reshape(NCORES, NT, G, K, G, K)
    atc = np.zeros((NCORES, NT, PP, CP), np.float32)
    cent = atc[:, :, :P, :G].reshape(NCORES, NT, G, K, G)
    for g in range(G):
        bd[:, :, g, :, g, :] = ATs[:, :, g]          # block-diagonal AT
        cent[:, :, g, :, g] = ATs[:, :, g, :, 0]     # center (t_local=0) col
    # device layout [PP, NT, .]
    at = np.ascontiguousarray(at.transpose(0, 2, 1, 3).astype(bf))
    atc = np.ascontiguousarray(atc.transpose(0, 2, 1, 3).astype(bf))

    # node-major x, tiled and padded: x_nm[p, i, f] = x[i*P + p, f], p < 125
    xp = np.zeros((NCORES, NT, PP, F0), np.float32)
    xtmp = np.zeros((NCORES, NT * P, F0), np.float32)
    xtmp[:, :GPC * K] = x.reshape(NCORES, GPC * K, F0)
    xp[:, :, :P, :] = xtmp.reshape(NCORES, NT, P, F0)
    x_nm = np.ascontiguousarray(xp.transpose(0, 2, 1, 3).astype(bf))

    w1p = np.ascontiguousarray(
        W1.reshape(2, 128, F1).transpose(1, 0, 2).astype(bf))  # [128, fic, fo]
    wl = np.ascontiguousarray(Wlin.reshape(2, 128).T.astype(bf))  # [128, foc]

    in_maps = []
    for c in range(NCORES):
        in_maps.append({
            "x": x_nm[c],
            "at": np.ascontiguousarray(at[c]),
            "atc": np.ascontiguousarray(atc[c]),
            "w0": np.ascontiguousarray(W0.astype(bf)),
            "w1": w1p,
            "wl": wl,
        })
    return in_maps


def _run(inputs, mode=None, trace=False):
    nc = _get_nc()
    in_maps = _host_prep(**inputs)
    res = run_bass_kernel_spmd(nc, in_maps, core_ids=list(range(NCORES)),
                               trace=trace)
    out = np.empty((B, 1), np.float32)
    for c in range(NCORES):
        vals = res.results[c]["out"].reshape(NT, CP)[:, :G].reshape(-1)
        out[c * GPC:(c + 1) * GPC, 0] = vals[:GPC]
    return out, res


def kernel(**inputs):
    out, _ = _run(inputs, trace=False)
    return out
